# revision 5
# baseline (speedup 1.0000x reference)
"""Trainium2 Bass kernel for nn_AgentAndNode_embedding (GIN message passing +
per-agent attention pooling), data-parallel over 8 NeuronCores.

Strategy
--------
* Shard by graph: 16 graphs x 1000 nodes per core (edges never cross graphs).
* segment_sum -> dense per-graph adjacency matmul on the PE:
    agg^T[c, i] = sum_s h[s, c] * AT[s, i],  AT[s, i] = #edges (src=s -> dst=i)
  AT is built host-side (bincount over the static edge list) and stored in
  fp8-e4m3 (integers 0..16 are exact); h is fp16 stationary, AT the fp8
  moving operand, f32 PSUM accumulation.
* GIN MLP in transposed layout (channels on partitions, weights stationary),
  b2 dropped (cancels inside BatchNorm).  BatchNorm stats via
  bn_stats/bn_aggr per tile, one [64,2] AllReduce per layer for the global
  batch stats.  The per-layer transpose pass (for the next layer's stationary
  operand) reads the *pre-BN* m, so it executes inside the AllReduce bubble;
  the BN affine+relu is then applied per graph in both layouts (ACT for the
  transposed copy, DVE+GpSimd for the natural copy) right before use.
* Attention without materializing k/v:
    u_am = f_m . (Wk_a q_a) / sqrt(K), w~ = exp(u/8) (bk.q / max-sub cancel),
    Z = sum w~, s~ = w~^T f, emb_a = (s~_a / Z_a) Wv_a + bv_a.
"""

import os
import numpy as np
import ml_dtypes

import concourse.bass as bass
import concourse.bacc as bacc
import concourse.tile as tile
from concourse import mybir
from concourse.bass_utils import run_bass_kernel_spmd
from concourse.masks import make_identity

FP16 = mybir.dt.float16
FP8 = mybir.dt.float8e4
F32 = mybir.dt.float32

NCORES = 8
G = 16          # graphs per core
NN = 1000       # nodes per graph
B = 128         # total graphs
CH = 64         # hidden
CIN = 2         # input channels
CSZ = 125       # nodes per src-chunk
NK = 8          # chunks per graph (8 * 125 = 1000)
NA = 10         # agents
AT_RES = 7      # graphs whose A^T stays SBUF-resident across layers
BN_EPS = 1e-5

AF = mybir.ActivationFunctionType
ALU = mybir.AluOpType

_PROG_CACHE = {}
LAST_RESULTS = None


def _build_program(at_dtype):
    nc = bacc.Bacc("TRN2", target_bir_lowering=False, debug=False,
                   num_devices=NCORES)

    at_d = nc.dram_tensor("at", [G, CSZ, NK, NN], at_dtype, kind="ExternalInput").ap()
    xnat_d = nc.dram_tensor("xnat", [CSZ, G, NK, CIN], FP16, kind="ExternalInput").ap()
    xT_d = nc.dram_tensor("xT", [CIN, G * NN], FP16, kind="ExternalInput").ap()
    w10_d = nc.dram_tensor("w10", [CIN, CH], FP16, kind="ExternalInput").ap()
    wpack_d = nc.dram_tensor("wpack", [CH, 8640], FP16, kind="ExternalInput").ap()
    cpack_d = nc.dram_tensor("cpack", [CH, 29], F32, kind="ExternalInput").ap()
    h3_d = nc.dram_tensor("h3", [G, NN, CH], F32, kind="ExternalOutput").ap()
    embT_d = nc.dram_tensor("embT", [CH, NA, G], F32, kind="ExternalOutput").ap()

    W1_OFF = {1: 0, 2: 64}
    W2_OFF = {0: 128, 1: 192, 2: 256}
    WKT_OFF = 320
    WV_OFF = 960
    WQP_OFF = 1600

    with tile.TileContext(nc) as tc:
        with tc.tile_pool(name="pers", bufs=1) as pers, \
             tc.tile_pool(name="stream", bufs=3) as stream, \
             tc.tile_pool(name="ypool", bufs=3) as ypool, \
             tc.tile_pool(name="small", bufs=2) as small, \
             tc.tile_pool(name="aggps", bufs=2, space="PSUM") as aggps, \
             tc.tile_pool(name="transps", bufs=2, space="PSUM") as transps, \
             tc.tile_pool(name="mlpps", bufs=2, space="PSUM") as mlpps, \
             tc.tile_pool(name="dram", bufs=2, space="DRAM") as dram:

            # ---------- constants / weights ----------
            ident = pers.tile([CH, CH], FP16)
            make_identity(nc, ident[:])
            ones = pers.tile([CSZ, 1], FP16)
            nc.vector.memset(ones[:], 1.0)
            eps_t = pers.tile([CH, 1], F32)
            nc.vector.memset(eps_t[:], BN_EPS)

            wpack = pers.tile([CH, 8640], FP16)
            nc.sync.dma_start(out=wpack[:], in_=wpack_d[:])
            cpack = pers.tile([CH, 29], F32)
            nc.sync.dma_start(out=cpack[:], in_=cpack_d[:])
            w10 = pers.tile([CIN, CH], FP16)
            nc.sync.dma_start(out=w10[:], in_=w10_d[:])
            xnat = pers.tile([CSZ, G, NK, CIN], FP16)
            nc.sync.dma_start(out=xnat[:], in_=xnat_d[:])

            at_res = pers.tile([CSZ, AT_RES, NK, NN], at_dtype)

            # ---------- persistent state ----------
            hT = pers.tile([CH, G, NN], FP16)
            mT = pers.tile([CH, G, NN], FP16)
            hnat = pers.tile([CSZ, G, NK, CH], FP16)

            def w1(l):
                return w10[:] if l == 0 else wpack[:, W1_OFF[l]:W1_OFF[l] + CH]

            def w2(l):
                return wpack[:, W2_OFF[l]:W2_OFF[l] + CH]

            # lazy per-graph finish of the previous layer's BN (affine+relu)
            # in both layouts; bcast = (a_b, c_b) broadcast tiles, af = (a, c).
            def affine_graph(g, bcast, af):
                a_b, c_b = bcast
                a_t, c_t = af
                nc.vector.tensor_tensor(out=hnat[:, g, :, :], in0=hnat[:, g, :, :],
                                        in1=a_b[:], op=ALU.mult)
                nc.vector.tensor_tensor(out=hnat[:, g, :, :], in0=hnat[:, g, :, :],
                                        in1=c_b[:], op=ALU.add)
                nc.gpsimd.tensor_scalar_max(out=hnat[:, g, :, :],
                                            in0=hnat[:, g, :, :], scalar1=0.0)
                nc.scalar.activation(out=hT[:, g, :], in_=mT[:, g, :],
                                     func=AF.Relu, bias=c_t[:], scale=a_t[:])

            bcast = af = None  # set after each layer's collective

            # ================= GIN layers =================
            for l in range(3):
                cin = CIN if l == 0 else CH
                hn = xnat if l == 0 else hnat
                stats = stream.tile([CH, 2 * G, 6], F32, tag="stats")
                at_views = {}

                def emit_fetch(g, l=l, at_views=at_views, bcast=bcast, af=af):
                    if l > 0:
                        affine_graph(g, bcast, af)
                    if g < AT_RES:
                        if l == 0:
                            nc.sync.dma_start(out=at_res[:, g, :, :], in_=at_d[g])
                        at_views[g] = at_res[:, g, :, :]
                    else:
                        t = stream.tile([CSZ, NK, NN], at_dtype, tag="at")
                        nc.sync.dma_start(out=t[:], in_=at_d[g])
                        at_views[g] = t[:]
                    if l == 0:
                        xg = stream.tile([CIN, NN], FP16, tag="xg")
                        nc.sync.dma_start(out=xg[:], in_=xT_d[:, g * NN:(g + 1) * NN])
                        at_views[(g, "x")] = xg[:]

                def emit_mms(g, l=l, cin=cin, hn=hn, at_views=at_views):
                    at_ap = at_views.pop(g)
                    aps = aggps.tile([cin, 2, 512], F32, tag="agg")
                    for k in range(NK):
                        for hf in range(2):
                            nc.tensor.matmul(
                                aps[:, hf, 0:500],
                                hn[:, g, k, :],
                                at_ap[:, k, hf * 500:(hf + 1) * 500],
                                start=(k == 0), stop=(k == NK - 1),
                            )
                    return aps

                def emit_rest(g, aps, l=l, cin=cin, at_views=at_views,
                              stats=stats):
                    hsrc = at_views.pop((g, "x")) if l == 0 else hT[:, g, :]
                    z = stream.tile([cin, NN], FP16, tag="z")
                    nc.vector.tensor_tensor(out=z[:], in0=aps[:, :, 0:500],
                                            in1=hsrc, op=ALU.add)
                    for t in range(2):
                        p1 = mlpps.tile([CH, 500], F32, tag="mlp")
                        nc.tensor.matmul(p1[:], w1(l), z[:, t * 500:(t + 1) * 500],
                                         start=True, stop=True)
                        y = ypool.tile([CH, 500], FP16, tag="y")
                        nc.scalar.activation(out=y[:], in_=p1[:], func=AF.Relu,
                                             bias=cpack[:, l:l + 1], scale=1.0)
                        p2 = mlpps.tile([CH, 500], F32, tag="mlp")
                        nc.tensor.matmul(p2[:], w2(l), y[:], start=True, stop=True)
                        nc.vector.bn_stats(out=stats[:, g * 2 + t, :], in_=p2[:])
                        nc.scalar.activation(out=mT[:, g, t * 500:(t + 1) * 500],
                                             in_=p2[:], func=AF.Copy)

                # software-pipelined graph loop (fetch 2 ahead, MMs 1 ahead)
                emit_fetch(0)
                emit_fetch(1)
                aps_prev = emit_mms(0)
                for g in range(G):
                    if g + 2 < G:
                        emit_fetch(g + 2)
                    aps_next = emit_mms(g + 1) if g + 1 < G else None
                    emit_rest(g, aps_prev)
                    aps_prev = aps_next

                # ---- launch cross-core BN stats reduction ASAP ----
                mv = small.tile([CH, 2], F32, tag="mv")
                nc.vector.bn_aggr(out=mv[:], in_=stats[:])
                red_in = small.tile([CH, 2], F32, tag="red_in")
                nc.vector.tensor_copy(out=red_in[:, 0:1], in_=mv[:, 0:1])
                msq = small.tile([CH, 1], F32, tag="msq")
                nc.vector.tensor_mul(out=msq[:], in0=mv[:, 0:1], in1=mv[:, 0:1])
                nc.vector.tensor_add(out=red_in[:, 1:2], in0=mv[:, 1:2], in1=msq[:])
                din = dram.tile([CH, 2], F32, tag="din")
                dout = dram.tile([CH, 2], F32, tag="dout")
                nc.sync.dma_start(out=din[:], in_=red_in[:])
                nc.gpsimd.collective_compute(
                    "AllReduce", ALU.add,
                    replica_groups=[list(range(NCORES))],
                    ins=[din.opt()], outs=[dout.opt()],
                )

                # ---- transpose pass (fills the collective bubble):
                # mT -> m_nat into hnat (affine applied lazily next layer)
                for g in range(G):
                    pt = transps.tile([CSZ, NK, CH], FP16, tag="pt")
                    for k in range(NK):
                        nc.tensor.transpose(
                            pt[:, k, :], mT[:, g, k * CSZ:(k + 1) * CSZ], ident[:])
                    nc.vector.tensor_copy(out=hnat[:, g, :, :], in_=pt[:])

                # ---- collective readback -> affine coefficients ----
                red = small.tile([CH, 2], F32, tag="red")
                nc.sync.dma_start(out=red[:], in_=dout[:])
                mu = small.tile([CH, 1], F32, tag="mu")
                nc.vector.tensor_scalar_mul(out=mu[:], in0=red[:, 0:1],
                                            scalar1=1.0 / NCORES)
                ex2 = small.tile([CH, 1], F32, tag="ex2")
                nc.vector.tensor_scalar_mul(out=ex2[:], in0=red[:, 1:2],
                                            scalar1=1.0 / NCORES)
                musq = small.tile([CH, 1], F32, tag="musq")
                nc.vector.tensor_mul(out=musq[:], in0=mu[:], in1=mu[:])
                var = small.tile([CH, 1], F32, tag="var")
                nc.vector.tensor_tensor(out=var[:], in0=ex2[:], in1=musq[:],
                                        op=ALU.subtract)
                sd = small.tile([CH, 1], F32, tag="sd")
                nc.scalar.activation(out=sd[:], in_=var[:], func=AF.Sqrt,
                                     bias=eps_t[:], scale=1.0)
                rstd = small.tile([CH, 1], F32, tag="rstd")
                nc.vector.reciprocal(out=rstd[:], in_=sd[:])
                a_t = small.tile([CH, 1], F32, tag="a_t")
                nc.vector.tensor_mul(out=a_t[:], in0=cpack[:, 3 + l:4 + l],
                                     in1=rstd[:])
                amu = small.tile([CH, 1], F32, tag="amu")
                nc.vector.tensor_mul(out=amu[:], in0=a_t[:], in1=mu[:])
                c_t = small.tile([CH, 1], F32, tag="c_t")
                nc.vector.tensor_tensor(out=c_t[:], in0=cpack[:, 6 + l:7 + l],
                                        in1=amu[:], op=ALU.subtract)
                # broadcast (a, c) across the 125 partitions via DRAM
                ac2 = small.tile([CH, 2], FP16, tag="ac2")
                nc.vector.tensor_copy(out=ac2[:, 0:1], in_=a_t[:])
                nc.vector.tensor_copy(out=ac2[:, 1:2], in_=c_t[:])
                acd = dram.tile([CH, 2], FP16, tag="acd")
                nc.sync.dma_start(out=acd[:], in_=ac2[:])
                acd_ap = acd[:]
                ab64 = small.tile([CSZ, CH], FP16, tag="ab64")
                cb64 = small.tile([CSZ, CH], FP16, tag="cb64")
                nc.sync.dma_start(out=ab64[:], in_=bass.AP(
                    tensor=acd_ap.tensor, offset=acd_ap.offset,
                    ap=[[0, CSZ], [2, CH]]))
                nc.sync.dma_start(out=cb64[:], in_=bass.AP(
                    tensor=acd_ap.tensor, offset=acd_ap.offset + 1,
                    ap=[[0, CSZ], [2, CH]]))
                a_b = small.tile([CSZ, NK, CH], FP16, tag="a_b")
                c_b = small.tile([CSZ, NK, CH], FP16, tag="c_b")
                for k in range(NK):
                    nc.vector.tensor_copy(out=a_b[:, k, :], in_=ab64[:])
                    nc.vector.tensor_copy(out=c_b[:, k, :], in_=cb64[:])
                bcast = (a_b, c_b)
                af = (a_t, c_t)

            # ================= finish layer 2 + outputs =================
            ghS32 = small.tile([CH, G], F32, tag="ghS32")
            scr = pers.tile([CH, NN], FP16)
            for g in range(G):
                affine_graph(g, bcast, af)
                nc.gpsimd.dma_start(
                    out=h3_d[g].rearrange("(k p) c -> p k c", p=CSZ),
                    in_=hnat[:, g, :, :],
                )
                nc.scalar.activation(out=scr[:], in_=hT[:, g, :], func=AF.Identity,
                                     scale=1.0, accum_out=ghS32[:, g:g + 1])
            ghS = small.tile([CH, G], FP16, tag="ghS")
            nc.vector.tensor_copy(out=ghS[:], in_=ghS32[:])

            # q_a^T [64k, 16g]: depot pieces first (only need hT), ghS last
            q_ps = mlpps.tile([CH, NA, G], F32, tag="mlp")
            for a in range(NA):
                for p in list(range(1, 11)) + [0]:
                    wq_ap = wpack[:, WQP_OFF + 64 * (a * 11 + p):
                                  WQP_OFF + 64 * (a * 11 + p) + 64]
                    rhs = ghS[:] if p == 0 else hT[:, :, p - 1]
                    nc.tensor.matmul(q_ps[:, a, :], wq_ap, rhs,
                                     start=(p == 1), stop=(p == 0))
            qT = pers.tile([CH, NA, G], FP16)
            for a in range(NA):
                nc.scalar.activation(out=qT[:, a, :], in_=q_ps[:, a, :],
                                     func=AF.Identity, bias=cpack[:, 9 + a:10 + a],
                                     scale=1.0)

            T_ps = mlpps.tile([CH, NA, G], F32, tag="mlp")
            for a in range(NA):
                nc.tensor.matmul(T_ps[:, a, :],
                                 wpack[:, WKT_OFF + 64 * a:WKT_OFF + 64 * a + 64],
                                 qT[:, a, :], start=True, stop=True)
            T_all = pers.tile([CH, NA, G], FP16)
            nc.vector.tensor_copy(out=T_all[:], in_=T_ps[:])

            Z_ps = aggps.tile([1, G, NA], F32, tag="agg")
            s_ps = aggps.tile([CH, G, NA], F32, tag="agg")

            def emit_u(g):
                up = transps.tile([CSZ, NK, NA], F32, tag="pt")
                for k in range(NK):
                    nc.tensor.matmul(up[:, k, :], hT[:, g, k * CSZ:(k + 1) * CSZ],
                                     T_all[:, :, g], start=True, stop=True)
                wt = stream.tile([CSZ, NK, NA], FP16, tag="wt")
                nc.scalar.activation(out=wt[:], in_=up[:], func=AF.Exp, scale=0.125)
                nc.vector.memset(wt[0:NA, 0, :], 0.0)
                return wt

            def emit_sz(g, wt):
                for k in range(NK):
                    nc.tensor.matmul(Z_ps[:, g, :], ones[:], wt[:, k, :],
                                     start=(k == 0), stop=(k == NK - 1))
                for k in range(NK):
                    nc.tensor.matmul(s_ps[:, g, :], hnat[:, g, k, :], wt[:, k, :],
                                     start=(k == 0), stop=(k == NK - 1))

            wt = emit_u(0)
            for g in range(G):
                nwt = emit_u(g + 1) if g + 1 < G else None
                emit_sz(g, wt)
                wt = nwt

            Zs = small.tile([1, G * NA], F32, tag="Zs")
            nc.vector.tensor_copy(out=Zs[:], in_=Z_ps[0:1, :, :])
            rz = small.tile([1, G * NA], F32, tag="rz")
            nc.vector.reciprocal(out=rz[:], in_=Zs[:])
            rzb = dram.tile([1, G * NA], F32, tag="rzb")
            nc.sync.dma_start(out=rzb[:], in_=rz[:])
            rzB = pers.tile([CH, G * NA], F32)
            rzb_ap = rzb[:]
            nc.sync.dma_start(out=rzB[:], in_=bass.AP(
                tensor=rzb_ap.tensor, offset=rzb_ap.offset,
                ap=[[0, CH]] + list(rzb_ap.ap[1:])))
            sT = pers.tile([CH, G, NA], FP16)
            nc.vector.tensor_tensor(out=sT[:], in0=s_ps[:],
                                    in1=rzB[:].rearrange("c (g a) -> c g a", g=G),
                                    op=ALU.mult)

            emb_ps = mlpps.tile([CH, NA, G], F32, tag="mlp")
            for a in range(NA):
                nc.tensor.matmul(emb_ps[:, a, :],
                                 wpack[:, WV_OFF + 64 * a:WV_OFF + 64 * a + 64],
                                 sT[:, :, a], start=True, stop=True)
            emb_sb = pers.tile([CH, NA, G], F32)
            for a in range(NA):
                nc.scalar.activation(out=emb_sb[:, a, :], in_=emb_ps[:, a, :],
                                     func=AF.Identity, bias=cpack[:, 19 + a:20 + a],
                                     scale=1.0)
            nc.sync.dma_start(out=embT_d[:], in_=emb_sb[:])

    nc.compile()
    return nc


def _prep_host(x, edge_src, edge_dst, gin_params, Wq, bq, Wk, bk, Wv, bv):
    x = np.asarray(x, np.float32)
    src = np.asarray(edge_src, np.int64)
    dst = np.asarray(edge_dst, np.int64)

    g_dst = dst // NN
    assert (src // NN == g_dst).all(), "edges must stay within graphs"
    s_loc = src - (src // NN) * NN
    d_loc = dst - g_dst * NN

    flat = (g_dst * NN + s_loc) * NN + d_loc
    per_core_at = []
    max_cnt = 0
    for c in range(NCORES):
        lo = c * G * NN
        sel = (g_dst >= c * G) & (g_dst < (c + 1) * G)
        sub = flat[sel] - lo * NN
        counts = np.bincount(sub, minlength=G * NN * NN)
        max_cnt = max(max_cnt, int(counts.max()))
        at = counts.reshape(G, NK, CSZ, NN).transpose(0, 2, 1, 3)  # [G,125,8,1000]
        per_core_at.append(np.ascontiguousarray(at.astype(np.float32)))

    at_dtype = FP8 if max_cnt <= 16 else FP16
    np_at = mybir.dt.np(at_dtype)
    per_core_at = [a.astype(np_at) for a in per_core_at]

    per_core_x = []
    for c in range(NCORES):
        xc = x[c * G * NN:(c + 1) * G * NN]
        xnat = np.ascontiguousarray(
            xc.reshape(G, NK, CSZ, CIN).transpose(2, 0, 1, 3)).astype(np.float16)
        xT = np.ascontiguousarray(xc.T).astype(np.float16)
        per_core_x.append((xnat, xT))

    gp = [[np.asarray(t, np.float32) for t in layer] for layer in gin_params]
    Wq = np.asarray(Wq, np.float32); bq = np.asarray(bq, np.float32)
    Wk = np.asarray(Wk, np.float32); Wv = np.asarray(Wv, np.float32)
    bv = np.asarray(bv, np.float32)

    w10 = gp[0][0].astype(np.float16)
    wcols = [gp[1][0], gp[2][0], gp[0][2], gp[1][2], gp[2][2]]
    for a in range(NA):
        wcols.append(Wk[a].T)
    for a in range(NA):
        wcols.append(Wv[a])
    for a in range(NA):
        wcols.append(Wq[a][:CH] / float(NN))
        for d in range(NA):
            wcols.append(Wq[a][CH + CH * d:CH + CH * (d + 1)])
    wpack = np.concatenate(wcols, axis=1).astype(np.float16)
    assert wpack.shape == (CH, 8640), wpack.shape

    ccols = [gp[0][1], gp[1][1], gp[2][1],
             gp[0][4], gp[1][4], gp[2][4],
             gp[0][5], gp[1][5], gp[2][5]]
    ccols = [c.reshape(CH, 1) for c in ccols]
    ccols.append(bq.T)
    ccols.append(bv.T)
    cpack = np.concatenate(ccols, axis=1).astype(np.float32)
    assert cpack.shape == (CH, 29), cpack.shape

    in_maps = []
    for c in range(NCORES):
        xnat, xT = per_core_x[c]
        in_maps.append({
            "at": per_core_at[c],
            "xnat": xnat,
            "xT": xT,
            "w10": w10,
            "wpack": wpack,
            "cpack": cpack,
        })
    return in_maps, at_dtype


def kernel(x, edge_src, edge_dst, gin_params, Wq, bq, Wk, bk, Wv, bv,
           n_nodes, n_batch):
    global LAST_RESULTS
    assert int(n_nodes) == NN and int(n_batch) == B

    in_maps, at_dtype = _prep_host(x, edge_src, edge_dst, gin_params,
                                   Wq, bq, Wk, bk, Wv, bv)

    key = str(at_dtype)
    if key not in _PROG_CACHE:
        _PROG_CACHE[key] = _build_program(at_dtype)
    nc = _PROG_CACHE[key]

    trace = bool(os.environ.get("BASS_TRACE"))
    res = run_bass_kernel_spmd(nc, in_maps, core_ids=list(range(NCORES)),
                               trace=trace)
    LAST_RESULTS = res

    h3 = np.concatenate([r["h3"] for r in res.results], axis=0)
    f = np.ascontiguousarray(h3[:, NA:, :])
    emb = np.concatenate(
        [r["embT"].transpose(2, 1, 0) for r in res.results], axis=0)
    return emb, f


# revision 7
# speedup vs baseline: 2.8888x; 2.8888x over previous
"""Trainium2 Bass kernel for nn_AgentAndNode_embedding (GIN message passing +
per-agent attention pooling), data-parallel over 8 NeuronCores.

Strategy
--------
* Shard by graph: 16 graphs x 1000 nodes per core (edges never cross graphs).
* segment_sum -> dense per-graph adjacency matmul on the PE:
    agg^T[c, i] = sum_s h[s, c] * AT[s, i],  AT[s, i] = #edges (src=s -> dst=i)
  AT is built host-side (bincount over the static edge list) and stored in
  fp8-e4m3 (integers 0..16 are exact); h is fp16 stationary, AT the fp8
  moving operand, f32 PSUM accumulation.
* GIN MLP in transposed layout (channels on partitions, weights stationary),
  b2 dropped (cancels inside BatchNorm).  BatchNorm stats via
  bn_stats/bn_aggr per tile, one [64,2] AllReduce per layer for the global
  batch stats.  The per-layer transpose pass (for the next layer's stationary
  operand) reads the *pre-BN* m, so it executes inside the AllReduce bubble;
  the BN affine+relu is then applied per graph in both layouts (ACT for the
  transposed copy, DVE+GpSimd for the natural copy) right before use.
* Attention without materializing k/v:
    u_am = f_m . (Wk_a q_a) / sqrt(K), w~ = exp(u/8) (bk.q / max-sub cancel),
    Z = sum w~, s~ = w~^T f, emb_a = (s~_a / Z_a) Wv_a + bv_a.
"""

import os
import numpy as np
import ml_dtypes

import concourse.bass as bass
import concourse.bacc as bacc
import concourse.tile as tile
from concourse import mybir
from concourse.bass_utils import run_bass_kernel_spmd
from concourse.masks import make_identity

FP16 = mybir.dt.float16
FP8 = mybir.dt.float8e4
F32 = mybir.dt.float32

NCORES = 8
G = 16          # graphs per core
NN = 1000       # nodes per graph
B = 128         # total graphs
CH = 64         # hidden
CIN = 2         # input channels
CSZ = 125       # nodes per src-chunk
NK = 8          # chunks per graph (8 * 125 = 1000)
NA = 10         # agents
AT_RES = 7      # graphs whose A^T stays SBUF-resident across layers
BN_EPS = 1e-5

AF = mybir.ActivationFunctionType
ALU = mybir.AluOpType

_PROG_CACHE = {}
LAST_RESULTS = None


def _build_program(at_dtype):
    nc = bacc.Bacc("TRN2", target_bir_lowering=False, debug=False,
                   num_devices=NCORES)

    at_d = nc.dram_tensor("at", [G, CSZ, NK, NN], at_dtype, kind="ExternalInput").ap()
    xnat_d = nc.dram_tensor("xnat", [CSZ, G, NK, CIN], FP16, kind="ExternalInput").ap()
    xT_d = nc.dram_tensor("xT", [CIN, G * NN], FP16, kind="ExternalInput").ap()
    w10_d = nc.dram_tensor("w10", [CIN, CH], FP16, kind="ExternalInput").ap()
    wpack_d = nc.dram_tensor("wpack", [CH, 8640], FP16, kind="ExternalInput").ap()
    cpack_d = nc.dram_tensor("cpack", [CH, 29], F32, kind="ExternalInput").ap()
    h3_d = nc.dram_tensor("h3", [G, NN, CH], F32, kind="ExternalOutput").ap()
    embT_d = nc.dram_tensor("embT", [CH, NA, G], F32, kind="ExternalOutput").ap()
    aout_d = nc.dram_tensor("aout", [CH, 1], F32, kind="ExternalOutput").ap()

    W1_OFF = {1: 0, 2: 64}
    W2_OFF = {0: 128, 1: 192, 2: 256}
    WKT_OFF = 320
    WV_OFF = 960
    WQP_OFF = 1600

    with tile.TileContext(nc) as tc:
        with tc.tile_pool(name="pers", bufs=1) as pers, \
             tc.tile_pool(name="stream", bufs=3) as stream, \
             tc.tile_pool(name="ypool", bufs=3) as ypool, \
             tc.tile_pool(name="small", bufs=2) as small, \
             tc.tile_pool(name="aggps", bufs=2, space="PSUM") as aggps, \
             tc.tile_pool(name="transps", bufs=2, space="PSUM") as transps, \
             tc.tile_pool(name="mlpps", bufs=2, space="PSUM") as mlpps, \
             tc.tile_pool(name="dram", bufs=2, space="DRAM") as dram:

            # ---------- constants / weights ----------
            ident = pers.tile([CH, CH], FP16)
            make_identity(nc, ident[:])
            ones = pers.tile([CSZ, 1], FP16)
            nc.vector.memset(ones[:], 1.0)
            eps_t = pers.tile([CH, 1], F32)
            nc.vector.memset(eps_t[:], BN_EPS)

            wpack = pers.tile([CH, 8640], FP16)
            nc.sync.dma_start(out=wpack[:], in_=wpack_d[:])
            cpack = pers.tile([CH, 29], F32)
            nc.sync.dma_start(out=cpack[:], in_=cpack_d[:])
            w10 = pers.tile([CIN, CH], FP16)
            nc.sync.dma_start(out=w10[:], in_=w10_d[:])
            xnat = pers.tile([CSZ, G, NK, CIN], FP16)
            nc.sync.dma_start(out=xnat[:], in_=xnat_d[:])

            at_res = pers.tile([CSZ, AT_RES, NK, NN], at_dtype)

            # ---------- persistent state ----------
            hT = pers.tile([CH, G, NN], FP16)
            mT = pers.tile([CH, G, NN], FP16)
            hnat = pers.tile([CSZ, G, NK, CH], FP16)

            def w1(l, lstate):
                return w10[:] if l == 0 else lstate[4][:]

            def w2(l):
                return wpack[:, W2_OFF[l]:W2_OFF[l] + CH]

            # lazy per-graph finish of the previous layer's BN.  With
            # gamma > 0 (host canonicalizes), h = a * n, n = relu(m + c/a);
            # the per-channel a-scale is folded into the consumers (W1 of the
            # next layer / attention weights / host f-scale), so only n is
            # materialized: nT via ACT (per-partition bias), n_nat via two
            # non-in-place DVE passes over the pre-transposed m_nat.
            def finish_graph(g, cp_b, cp_t):
                tmp = stream.tile([CSZ, NK, CH], FP16, tag="afftmp")
                nc.vector.tensor_tensor(out=tmp[:], in0=hnat[:, g, :, :],
                                        in1=cp_b[:], op=ALU.add)
                nc.vector.tensor_scalar_max(out=hnat[:, g, :, :], in0=tmp[:],
                                            scalar1=0.0)
                nc.scalar.activation(out=hT[:, g, :], in_=mT[:, g, :],
                                     func=AF.Relu, bias=cp_t[:], scale=1.0)

            lstate = None  # (cp_b, cp_t, a_t, c_t, w1_eff) from previous layer

            # ================= GIN layers =================
            for l in range(3):
                cin = CIN if l == 0 else CH
                hn = xnat if l == 0 else hnat
                stats = stream.tile([CH, 2 * G, 6], F32, tag="stats")
                at_views = {}

                def emit_fetch(g, l=l, at_views=at_views, lstate=lstate):
                    if l > 0:
                        finish_graph(g, lstate[0], lstate[1])
                    if g < AT_RES:
                        if l == 0:
                            nc.sync.dma_start(out=at_res[:, g, :, :], in_=at_d[g])
                        at_views[g] = at_res[:, g, :, :]
                    else:
                        t = stream.tile([CSZ, NK, NN], at_dtype, tag="at")
                        nc.sync.dma_start(out=t[:], in_=at_d[g])
                        at_views[g] = t[:]
                    if l == 0:
                        xg = stream.tile([CIN, NN], FP16, tag="xg")
                        nc.sync.dma_start(out=xg[:], in_=xT_d[:, g * NN:(g + 1) * NN])
                        at_views[(g, "x")] = xg[:]

                def emit_mms(g, l=l, cin=cin, hn=hn, at_views=at_views):
                    at_ap = at_views.pop(g)
                    aps = aggps.tile([cin, 2, 512], F32, tag="agg")
                    for k in range(NK):
                        for hf in range(2):
                            nc.tensor.matmul(
                                aps[:, hf, 0:500],
                                hn[:, g, k, :],
                                at_ap[:, k, hf * 500:(hf + 1) * 500],
                                start=(k == 0), stop=(k == NK - 1),
                            )
                    return aps

                def emit_rest(g, aps, l=l, cin=cin, at_views=at_views,
                              stats=stats, lstate=lstate):
                    hsrc = at_views.pop((g, "x")) if l == 0 else hT[:, g, :]
                    z = stream.tile([cin, NN], FP16, tag="z")
                    nc.vector.tensor_tensor(out=z[:], in0=aps[:, :, 0:500],
                                            in1=hsrc, op=ALU.add)
                    for t in range(2):
                        p1 = mlpps.tile([CH, 500], F32, tag="mlp")
                        nc.tensor.matmul(p1[:], w1(l, lstate),
                                         z[:, t * 500:(t + 1) * 500],
                                         start=True, stop=True)
                        y = ypool.tile([CH, 500], FP16, tag="y")
                        nc.scalar.activation(out=y[:], in_=p1[:], func=AF.Relu,
                                             bias=cpack[:, l:l + 1], scale=1.0)
                        p2 = mlpps.tile([CH, 500], F32, tag="mlp")
                        nc.tensor.matmul(p2[:], w2(l), y[:], start=True, stop=True)
                        nc.vector.bn_stats(out=stats[:, g * 2 + t, :], in_=p2[:])
                        nc.scalar.activation(out=mT[:, g, t * 500:(t + 1) * 500],
                                             in_=p2[:], func=AF.Copy)

                # software-pipelined graph loop (fetch 2 ahead, MMs 1 ahead)
                emit_fetch(0)
                emit_fetch(1)
                aps_prev = emit_mms(0)
                for g in range(G):
                    if g + 2 < G:
                        emit_fetch(g + 2)
                    aps_next = emit_mms(g + 1) if g + 1 < G else None
                    emit_rest(g, aps_prev)
                    aps_prev = aps_next

                # ---- launch cross-core BN stats reduction ASAP ----
                mv = small.tile([CH, 2], F32, tag="mv")
                nc.vector.bn_aggr(out=mv[:], in_=stats[:])
                red_in = small.tile([CH, 2], F32, tag="red_in")
                nc.vector.tensor_copy(out=red_in[:, 0:1], in_=mv[:, 0:1])
                msq = small.tile([CH, 1], F32, tag="msq")
                nc.vector.tensor_mul(out=msq[:], in0=mv[:, 0:1], in1=mv[:, 0:1])
                nc.vector.tensor_add(out=red_in[:, 1:2], in0=mv[:, 1:2], in1=msq[:])
                din = dram.tile([CH, 2], F32, tag="din")
                dout = dram.tile([CH, 2], F32, tag="dout")
                nc.sync.dma_start(out=din[:], in_=red_in[:])
                nc.gpsimd.collective_compute(
                    "AllReduce", ALU.add,
                    replica_groups=[list(range(NCORES))],
                    ins=[din.opt()], outs=[dout.opt()],
                )

                # ---- transpose pass (fills the collective bubble):
                # mT -> m_nat into hnat (affine applied lazily next layer)
                for g in range(G):
                    pt = transps.tile([CSZ, NK, CH], FP16, tag="pt")
                    for k in range(NK):
                        nc.tensor.transpose(
                            pt[:, k, :], mT[:, g, k * CSZ:(k + 1) * CSZ], ident[:])
                    nc.vector.tensor_copy(out=hnat[:, g, :, :], in_=pt[:])

                # ---- collective readback -> affine coefficients ----
                red = small.tile([CH, 2], F32, tag="red")
                nc.sync.dma_start(out=red[:], in_=dout[:])
                mu = small.tile([CH, 1], F32, tag="mu")
                nc.vector.tensor_scalar_mul(out=mu[:], in0=red[:, 0:1],
                                            scalar1=1.0 / NCORES)
                ex2 = small.tile([CH, 1], F32, tag="ex2")
                nc.vector.tensor_scalar_mul(out=ex2[:], in0=red[:, 1:2],
                                            scalar1=1.0 / NCORES)
                musq = small.tile([CH, 1], F32, tag="musq")
                nc.vector.tensor_mul(out=musq[:], in0=mu[:], in1=mu[:])
                var = small.tile([CH, 1], F32, tag="var")
                nc.vector.tensor_tensor(out=var[:], in0=ex2[:], in1=musq[:],
                                        op=ALU.subtract)
                sd = small.tile([CH, 1], F32, tag="sd")
                nc.scalar.activation(out=sd[:], in_=var[:], func=AF.Sqrt,
                                     bias=eps_t[:], scale=1.0)
                rstd = small.tile([CH, 1], F32, tag="rstd")
                nc.vector.reciprocal(out=rstd[:], in_=sd[:])
                a_t = small.tile([CH, 1], F32, tag="a_t")
                nc.vector.tensor_mul(out=a_t[:], in0=cpack[:, 3 + l:4 + l],
                                     in1=rstd[:])
                amu = small.tile([CH, 1], F32, tag="amu")
                nc.vector.tensor_mul(out=amu[:], in0=a_t[:], in1=mu[:])
                c_t = small.tile([CH, 1], F32, tag="c_t")
                nc.vector.tensor_tensor(out=c_t[:], in0=cpack[:, 6 + l:7 + l],
                                        in1=amu[:], op=ALU.subtract)
                # c' = c / a; broadcast across the 125 partitions via DRAM
                ra = small.tile([CH, 1], F32, tag="ra")
                nc.vector.reciprocal(out=ra[:], in_=a_t[:])
                cp_t = small.tile([CH, 1], F32, tag="cp_t")
                nc.vector.tensor_mul(out=cp_t[:], in0=c_t[:], in1=ra[:])
                cp16 = small.tile([CH, 1], FP16, tag="cp16")
                nc.vector.tensor_copy(out=cp16[:], in_=cp_t[:])
                acd = dram.tile([CH, 1], FP16, tag="acd")
                nc.sync.dma_start(out=acd[:], in_=cp16[:])
                acd_ap = acd[:]
                cb64 = small.tile([CSZ, CH], FP16, tag="cb64")
                nc.sync.dma_start(out=cb64[:], in_=bass.AP(
                    tensor=acd_ap.tensor, offset=acd_ap.offset,
                    ap=[[0, CSZ], [1, CH]]))
                cp_b = small.tile([CSZ, NK, CH], FP16, tag="cp_b")
                for k in range(NK):
                    nc.vector.tensor_copy(out=cp_b[:, k, :], in_=cb64[:])
                if l < 2:
                    w1s = small.tile([CH, CH], FP16, tag="w1s")
                    nc.vector.tensor_scalar_mul(
                        out=w1s[:], in0=wpack[:, W1_OFF[l + 1]:W1_OFF[l + 1] + CH],
                        scalar1=a_t[:])
                else:
                    w1s = None
                    nc.sync.dma_start(out=aout_d[:], in_=a_t[:])
                lstate = (cp_b, cp_t, a_t, c_t, w1s)

            # ================= finish layer 2 + outputs =================
            ghS32 = small.tile([CH, G], F32, tag="ghS32")
            scr = pers.tile([CH, NN], FP16)
            cp_b2, cp_t2, a2_t, c2_t, _ = lstate
            for g in range(G):
                tmp = stream.tile([CSZ, NK, CH], FP16, tag="afftmp")
                nc.vector.tensor_tensor(out=tmp[:], in0=hnat[:, g, :, :],
                                        in1=cp_b2[:], op=ALU.add)
                nc.vector.tensor_scalar_max(out=hnat[:, g, :, :], in0=tmp[:],
                                            scalar1=0.0)
                nc.scalar.activation(out=hT[:, g, :], in_=mT[:, g, :],
                                     func=AF.Relu, bias=c2_t[:], scale=a2_t[:])
                nc.gpsimd.dma_start(
                    out=h3_d[g].rearrange("(k p) c -> p k c", p=CSZ),
                    in_=hnat[:, g, :, :],
                )
                nc.scalar.activation(out=scr[:], in_=hT[:, g, :], func=AF.Identity,
                                     scale=1.0, accum_out=ghS32[:, g:g + 1])
            ghS = small.tile([CH, G], FP16, tag="ghS")
            nc.vector.tensor_copy(out=ghS[:], in_=ghS32[:])

            # q_a^T [64k, 16g]: depot pieces first (only need hT), ghS last
            q_ps = mlpps.tile([CH, NA, G], F32, tag="mlp")
            for a in range(NA):
                for p in list(range(1, 11)) + [0]:
                    wq_ap = wpack[:, WQP_OFF + 64 * (a * 11 + p):
                                  WQP_OFF + 64 * (a * 11 + p) + 64]
                    rhs = ghS[:] if p == 0 else hT[:, :, p - 1]
                    nc.tensor.matmul(q_ps[:, a, :], wq_ap, rhs,
                                     start=(p == 1), stop=(p == 0))
            qT = pers.tile([CH, NA, G], FP16)
            for a in range(NA):
                nc.scalar.activation(out=qT[:, a, :], in_=q_ps[:, a, :],
                                     func=AF.Identity, bias=cpack[:, 9 + a:10 + a],
                                     scale=1.0)

            T_ps = mlpps.tile([CH, NA, G], F32, tag="mlp")
            for a in range(NA):
                nc.tensor.matmul(T_ps[:, a, :],
                                 wpack[:, WKT_OFF + 64 * a:WKT_OFF + 64 * a + 64],
                                 qT[:, a, :], start=True, stop=True)
            T_all = pers.tile([CH, NA, G], FP16)
            nc.vector.tensor_copy(out=T_all[:], in_=T_ps[:])

            Z_ps = aggps.tile([1, G, NA], F32, tag="agg")
            s_ps = aggps.tile([CH, G, NA], F32, tag="agg")

            def emit_u(g):
                up = transps.tile([CSZ, NK, NA], F32, tag="pt")
                for k in range(NK):
                    nc.tensor.matmul(up[:, k, :], hT[:, g, k * CSZ:(k + 1) * CSZ],
                                     T_all[:, :, g], start=True, stop=True)
                wt = stream.tile([CSZ, NK, NA], FP16, tag="wt")
                nc.scalar.activation(out=wt[:], in_=up[:], func=AF.Exp, scale=0.125)
                nc.vector.memset(wt[0:NA, 0, :], 0.0)
                return wt

            def emit_sz(g, wt):
                for k in range(NK):
                    nc.tensor.matmul(Z_ps[:, g, :], ones[:], wt[:, k, :],
                                     start=(k == 0), stop=(k == NK - 1))
                for k in range(NK):
                    nc.tensor.matmul(s_ps[:, g, :], hnat[:, g, k, :], wt[:, k, :],
                                     start=(k == 0), stop=(k == NK - 1))

            wt = emit_u(0)
            for g in range(G):
                nwt = emit_u(g + 1) if g + 1 < G else None
                emit_sz(g, wt)
                wt = nwt

            Zs = small.tile([1, G * NA], F32, tag="Zs")
            nc.vector.tensor_copy(out=Zs[:], in_=Z_ps[0:1, :, :])
            rz = small.tile([1, G * NA], F32, tag="rz")
            nc.vector.reciprocal(out=rz[:], in_=Zs[:])
            rzb = dram.tile([1, G * NA], F32, tag="rzb")
            nc.sync.dma_start(out=rzb[:], in_=rz[:])
            rzB = pers.tile([CH, G * NA], F32)
            rzb_ap = rzb[:]
            nc.sync.dma_start(out=rzB[:], in_=bass.AP(
                tensor=rzb_ap.tensor, offset=rzb_ap.offset,
                ap=[[0, CH]] + list(rzb_ap.ap[1:])))
            sTa = pers.tile([CH, G, NA], F32)
            nc.vector.tensor_scalar_mul(out=sTa[:], in0=s_ps[:], scalar1=a2_t[:])
            sT = pers.tile([CH, G, NA], FP16)
            nc.vector.tensor_tensor(out=sT[:], in0=sTa[:],
                                    in1=rzB[:].rearrange("c (g a) -> c g a", g=G),
                                    op=ALU.mult)

            emb_ps = mlpps.tile([CH, NA, G], F32, tag="mlp")
            for a in range(NA):
                nc.tensor.matmul(emb_ps[:, a, :],
                                 wpack[:, WV_OFF + 64 * a:WV_OFF + 64 * a + 64],
                                 sT[:, :, a], start=True, stop=True)
            emb_sb = pers.tile([CH, NA, G], F32)
            for a in range(NA):
                nc.scalar.activation(out=emb_sb[:, a, :], in_=emb_ps[:, a, :],
                                     func=AF.Identity, bias=cpack[:, 19 + a:20 + a],
                                     scale=1.0)
            nc.sync.dma_start(out=embT_d[:], in_=emb_sb[:])

    nc.compile()
    return nc


def _prep_host(x, edge_src, edge_dst, gin_params, Wq, bq, Wk, bk, Wv, bv):
    x = np.asarray(x, np.float32)
    src = np.asarray(edge_src, np.int64)
    dst = np.asarray(edge_dst, np.int64)

    g_dst = dst // NN
    assert (src // NN == g_dst).all(), "edges must stay within graphs"
    s_loc = src - (src // NN) * NN
    d_loc = dst - g_dst * NN

    flat = (g_dst * NN + s_loc) * NN + d_loc
    per_core_at = []
    max_cnt = 0
    for c in range(NCORES):
        lo = c * G * NN
        sel = (g_dst >= c * G) & (g_dst < (c + 1) * G)
        sub = flat[sel] - lo * NN
        counts = np.bincount(sub, minlength=G * NN * NN)
        max_cnt = max(max_cnt, int(counts.max()))
        at = counts.reshape(G, NK, CSZ, NN).transpose(0, 2, 1, 3)  # [G,125,8,1000]
        per_core_at.append(np.ascontiguousarray(at.astype(np.float32)))

    at_dtype = FP8 if max_cnt <= 16 else FP16
    np_at = mybir.dt.np(at_dtype)
    per_core_at = [a.astype(np_at) for a in per_core_at]

    per_core_x = []
    for c in range(NCORES):
        xc = x[c * G * NN:(c + 1) * G * NN]
        xnat = np.ascontiguousarray(
            xc.reshape(G, NK, CSZ, CIN).transpose(2, 0, 1, 3)).astype(np.float16)
        xT = np.ascontiguousarray(xc.T).astype(np.float16)
        per_core_x.append((xnat, xT))

    gp = [[np.asarray(t, np.float32).copy() for t in layer] for layer in gin_params]
    # canonicalize gamma > 0: flipping the sign of W2's column c flips m[:, c],
    # and (gamma, mu) flip with it, leaving BN output identical.  Needed so
    # relu(a*m + c) = a * relu(m + c/a) holds with a = gamma*rstd > 0.
    for layer in gp:
        gamma = layer[4]
        assert np.all(gamma != 0.0), "zero BN gamma not supported"
        neg = gamma < 0
        if neg.any():
            layer[2][:, neg] *= -1.0   # W2 columns
            layer[3][neg] *= -1.0      # b2 (unused, for consistency)
            layer[4][neg] *= -1.0      # gamma
    Wq = np.asarray(Wq, np.float32); bq = np.asarray(bq, np.float32)
    Wk = np.asarray(Wk, np.float32); Wv = np.asarray(Wv, np.float32)
    bv = np.asarray(bv, np.float32)

    w10 = gp[0][0].astype(np.float16)
    wcols = [gp[1][0], gp[2][0], gp[0][2], gp[1][2], gp[2][2]]
    for a in range(NA):
        wcols.append(Wk[a].T)
    for a in range(NA):
        wcols.append(Wv[a])
    for a in range(NA):
        wcols.append(Wq[a][:CH] / float(NN))
        for d in range(NA):
            wcols.append(Wq[a][CH + CH * d:CH + CH * (d + 1)])
    wpack = np.concatenate(wcols, axis=1).astype(np.float16)
    assert wpack.shape == (CH, 8640), wpack.shape

    ccols = [gp[0][1], gp[1][1], gp[2][1],
             gp[0][4], gp[1][4], gp[2][4],
             gp[0][5], gp[1][5], gp[2][5]]
    ccols = [c.reshape(CH, 1) for c in ccols]
    ccols.append(bq.T)
    ccols.append(bv.T)
    cpack = np.concatenate(ccols, axis=1).astype(np.float32)
    assert cpack.shape == (CH, 29), cpack.shape

    in_maps = []
    for c in range(NCORES):
        xnat, xT = per_core_x[c]
        in_maps.append({
            "at": per_core_at[c],
            "xnat": xnat,
            "xT": xT,
            "w10": w10,
            "wpack": wpack,
            "cpack": cpack,
        })
    return in_maps, at_dtype


def kernel(x, edge_src, edge_dst, gin_params, Wq, bq, Wk, bk, Wv, bv,
           n_nodes, n_batch):
    global LAST_RESULTS
    assert int(n_nodes) == NN and int(n_batch) == B

    in_maps, at_dtype = _prep_host(x, edge_src, edge_dst, gin_params,
                                   Wq, bq, Wk, bk, Wv, bv)

    key = str(at_dtype)
    if key not in _PROG_CACHE:
        _PROG_CACHE[key] = _build_program(at_dtype)
    nc = _PROG_CACHE[key]

    trace = bool(os.environ.get("BASS_TRACE"))
    res = run_bass_kernel_spmd(nc, in_maps, core_ids=list(range(NCORES)),
                               trace=trace)
    LAST_RESULTS = res

    h3 = np.concatenate([r["h3"] for r in res.results], axis=0)
    a2 = res.results[0]["aout"].reshape(CH).astype(np.float32)
    f = np.ascontiguousarray(h3[:, NA:, :]) * a2[None, None, :]
    emb = np.concatenate(
        [r["embT"].transpose(2, 1, 0) for r in res.results], axis=0)
    return emb, f
